# revision 4
# baseline (speedup 1.0000x reference)
"""Trainium2 Bass kernel for DiagonalMatrixModel: out = x * diagonal[None, :].

x: [8192, 4096] f32, diagonal: [4096] f32 -> out: [8192, 4096] f32.
Data-parallel across 8 NeuronCores: each core handles 1024 rows; the
diagonal is replicated. Per core the kernel is purely DMA-bound
(16 MiB in + 16 MiB out at ~360-425 GB/s per-core HBM bandwidth).

Strategy per core:
  - broadcast-DMA the diagonal into a [128, 4096] SBUF tile once
    (stride-0 partition AP via SWDGE)
  - loop over 8 row-tiles [128, 4096] (2 MiB each): HWDGE DMA in,
    in-place vector-engine multiply against the diag tile, HWDGE DMA out
  - tile pool with multiple bufs so in/compute/out fully overlap
"""

import numpy as np

import bass_rust
import concourse.bass as bass
import concourse.mybir as mybir
from concourse.bass_utils import run_bass_kernel_spmd
from concourse.tile import TileContext
from concourse.vector_clock import ScopedClock


def _split_multiwait_instructions(nc):
    """Walrus codegen in this container accepts at most ONE sync-wait per
    instruction ("Too many sync wait commands", CoreV3GenImpl setupSyncWait).
    Tile's semaphore assignment freely attaches several.  Rewrite every
    basic block: excess waits move onto same-engine NOPs inserted
    immediately before the overloaded instruction (the engine stalls on the
    NOPs first, so the synchronization semantics are identical).
    """
    fn = nc.m.functions[0]
    for bb in fn.blocks:
        insts = list(bb.instructions)
        out = []
        changed = False
        for inst in insts:
            si = inst.sync_info
            waits = list(si.on_wait) if si and si.on_wait else []
            if len(waits) > 1:
                changed = True
                eng = nc.engines[inst.engine]
                for w in waits[:-1]:
                    nop = eng.nop()
                    # eng.nop() appended itself to the current bb; steal it.
                    cur = nc.cur_bb.bb
                    lst = list(cur.instructions)
                    assert lst[-1].name == nop.ins.name
                    cur.instructions = lst[:-1]
                    stolen = nop.ins
                    stolen.sync_info = bass_rust.SyncInfo(on_wait=[w], on_update=[])
                    out.append(stolen)
                si.on_wait = waits[-1:]
            out.append(inst)
        if changed:
            bb.instructions = out


def _patched_drain_and_barrier(self, tick_clock, wait_clock):
    """Replacement for TileContext._drain_and_barrier.

    Identical to stock, plus: runs _split_multiwait_instructions at the end
    (at that point every instruction, including this tail, is committed to
    the basic blocks and no further sync rewriting happens).
    """
    nc = self.nc
    drain_inst = nc.sync.drain()
    wait_clock.add_sem_waits(
        drain_inst.ins, ScopedClock({None: tick_clock.global_clock})
    )
    nc.all_engine_barrier()
    popped = nc._tile_sem_poison_stack.pop()
    assert popped is self._sem_poison
    nc.clear_and_free_semaphores(list(self.sems.allocated().values()))
    nc.all_engine_barrier()
    _split_multiwait_instructions(nc)


TileContext._drain_and_barrier = _patched_drain_and_barrier

BATCH = 8192
SIZE = 4096
N_CORES = 8
ROWS = BATCH // N_CORES  # 1024 rows per core
P = 128
N_TILES = ROWS // P  # 8

_CACHE: dict = {}


def _build() -> bass.Bass:
    nc = bass.Bass("TRN2")
    x = nc.dram_tensor("x", [ROWS, SIZE], mybir.dt.float32, kind="ExternalInput")
    dg = nc.dram_tensor("diagonal", [SIZE], mybir.dt.float32, kind="ExternalInput")
    out = nc.dram_tensor("out", [ROWS, SIZE], mybir.dt.float32, kind="ExternalOutput")

    with TileContext(nc) as tc:
        with (
            tc.tile_pool(name="const", bufs=1) as cpool,
            tc.tile_pool(name="work", bufs=8) as wpool,
        ):
            # Broadcast the diagonal across all 128 partitions with a
            # stride-0 HWDGE DMA (SWDGE takes ~25us to land; HWDGE is fast).
            dtile = cpool.tile([P, SIZE], mybir.dt.float32)
            nc.sync.dma_start(out=dtile[:], in_=dg[:].partition_broadcast(P))
            for i in range(N_TILES):
                xt = wpool.tile([P, SIZE], mybir.dt.float32)
                # Loads issue from the SP HWDGE ring, stores from the ACT
                # ring: store waits never block later loads.
                nc.sync.dma_start(out=xt[:], in_=x[i * P : (i + 1) * P, :])
                nc.vector.tensor_mul(xt[:], xt[:], dtile[:])
                nc.scalar.dma_start(out=out[i * P : (i + 1) * P, :], in_=xt[:])
    return nc


def kernel(x: np.ndarray, diagonal: np.ndarray) -> np.ndarray:
    if "nc" not in _CACHE:
        _CACHE["nc"] = _build()
    nc = _CACHE["nc"]

    x = np.ascontiguousarray(np.asarray(x, dtype=np.float32))
    diagonal = np.ascontiguousarray(np.asarray(diagonal, dtype=np.float32))

    shards = np.split(x, N_CORES, axis=0)
    in_maps = [{"x": s, "diagonal": diagonal} for s in shards]
    res = run_bass_kernel_spmd(nc, in_maps, list(range(N_CORES))).results
    return np.concatenate([r["out"] for r in res], axis=0)


# revision 5
# speedup vs baseline: 1.1714x; 1.1714x over previous
"""Trainium2 Bass kernel for DiagonalMatrixModel: out = x * diagonal[None, :].

x: [8192, 4096] f32, diagonal: [4096] f32 -> out: [8192, 4096] f32.
Data-parallel across 8 NeuronCores: each core handles 1024 rows; the
diagonal is replicated. Per core the kernel is purely DMA-bound
(16 MiB in + 16 MiB out at ~360-425 GB/s per-core HBM bandwidth).

Strategy per core:
  - broadcast-DMA the diagonal into a [128, 4096] SBUF tile once
    (stride-0 partition AP via SWDGE)
  - loop over 8 row-tiles [128, 4096] (2 MiB each): HWDGE DMA in,
    in-place vector-engine multiply against the diag tile, HWDGE DMA out
  - tile pool with multiple bufs so in/compute/out fully overlap
"""

import numpy as np

import bass_rust
import concourse.bass as bass
import concourse.mybir as mybir
from concourse.bass_utils import run_bass_kernel_spmd
from concourse.tile import TileContext
from concourse.vector_clock import ScopedClock


def _split_multiwait_instructions(nc):
    """Walrus codegen in this container accepts at most ONE sync-wait per
    instruction ("Too many sync wait commands", CoreV3GenImpl setupSyncWait).
    Tile's semaphore assignment freely attaches several.  Rewrite every
    basic block: excess waits move onto same-engine NOPs inserted
    immediately before the overloaded instruction (the engine stalls on the
    NOPs first, so the synchronization semantics are identical).
    """
    fn = nc.m.functions[0]
    for bb in fn.blocks:
        insts = list(bb.instructions)
        out = []
        changed = False
        for inst in insts:
            si = inst.sync_info
            waits = list(si.on_wait) if si and si.on_wait else []
            if len(waits) > 1:
                changed = True
                eng = nc.engines[inst.engine]
                for w in waits[:-1]:
                    nop = eng.nop()
                    # eng.nop() appended itself to the current bb; steal it.
                    cur = nc.cur_bb.bb
                    lst = list(cur.instructions)
                    assert lst[-1].name == nop.ins.name
                    cur.instructions = lst[:-1]
                    stolen = nop.ins
                    stolen.sync_info = bass_rust.SyncInfo(on_wait=[w], on_update=[])
                    out.append(stolen)
                si.on_wait = waits[-1:]
            out.append(inst)
        if changed:
            bb.instructions = out


def _patched_drain_and_barrier(self, tick_clock, wait_clock):
    """Replacement for TileContext._drain_and_barrier.

    Identical to stock, plus: runs _split_multiwait_instructions at the end
    (at that point every instruction, including this tail, is committed to
    the basic blocks and no further sync rewriting happens).
    """
    nc = self.nc
    drain_inst = nc.sync.drain()
    wait_clock.add_sem_waits(
        drain_inst.ins, ScopedClock({None: tick_clock.global_clock})
    )
    nc.all_engine_barrier()
    popped = nc._tile_sem_poison_stack.pop()
    assert popped is self._sem_poison
    nc.clear_and_free_semaphores(list(self.sems.allocated().values()))
    nc.all_engine_barrier()
    _split_multiwait_instructions(nc)


TileContext._drain_and_barrier = _patched_drain_and_barrier

BATCH = 8192
SIZE = 4096
N_CORES = 8
ROWS = BATCH // N_CORES  # 1024 rows per core
P = 128
N_TILES = ROWS // P  # 8

_CACHE: dict = {}


def _build() -> bass.Bass:
    nc = bass.Bass("TRN2")
    x = nc.dram_tensor("x", [ROWS, SIZE], mybir.dt.float32, kind="ExternalInput")
    dg = nc.dram_tensor("diagonal", [SIZE], mybir.dt.float32, kind="ExternalInput")
    out = nc.dram_tensor("out", [ROWS, SIZE], mybir.dt.float32, kind="ExternalOutput")

    with TileContext(nc) as tc:
        with (
            tc.tile_pool(name="const", bufs=1) as cpool,
            tc.tile_pool(name="work", bufs=N_TILES) as wpool,
        ):
            # Broadcast the diagonal across all 128 partitions with a
            # stride-0 HWDGE DMA (SWDGE takes ~25us to land; HWDGE is fast).
            dtile = cpool.tile([P, SIZE], mybir.dt.float32)
            nc.sync.dma_start(out=dtile[:], in_=dg[:].partition_broadcast(P))
            # All loads issue first, split across the two HWDGE rings (SP
            # and ACT), so both rings stream input at full rate with no
            # store-waits ahead of them in ring FIFO.  Stores follow behind
            # on the same rings; by the time the rings drain down to a
            # store, its tensor_mul has long completed.
            tiles = []
            for i in range(N_TILES):
                xt = wpool.tile([P, SIZE], mybir.dt.float32)
                eng = nc.sync if i % 2 == 0 else nc.scalar
                eng.dma_start(out=xt[:], in_=x[i * P : (i + 1) * P, :])
                tiles.append(xt)
            for i, xt in enumerate(tiles):
                nc.vector.tensor_mul(xt[:], xt[:], dtile[:])
                eng = nc.scalar if i % 2 == 0 else nc.sync
                eng.dma_start(out=out[i * P : (i + 1) * P, :], in_=xt[:])
    return nc


def kernel(x: np.ndarray, diagonal: np.ndarray) -> np.ndarray:
    if "nc" not in _CACHE:
        _CACHE["nc"] = _build()
    nc = _CACHE["nc"]

    x = np.ascontiguousarray(np.asarray(x, dtype=np.float32))
    diagonal = np.ascontiguousarray(np.asarray(diagonal, dtype=np.float32))

    shards = np.split(x, N_CORES, axis=0)
    in_maps = [{"x": s, "diagonal": diagonal} for s in shards]
    res = run_bass_kernel_spmd(nc, in_maps, list(range(N_CORES))).results
    return np.concatenate([r["out"] for r in res], axis=0)


# revision 9
# speedup vs baseline: 1.2005x; 1.0249x over previous
"""Trainium2 Bass kernel for DiagonalMatrixModel: out = x * diagonal[None, :].

x: [8192, 4096] f32, diagonal: [4096] f32 -> out: [8192, 4096] f32.
Data-parallel across 8 NeuronCores: each core handles 1024 rows; the
diagonal is replicated. Per core the kernel is purely DMA-bound
(16 MiB in + 16 MiB out at ~360-425 GB/s per-core HBM bandwidth).

Strategy per core:
  - broadcast-DMA the diagonal into a [128, 4096] SBUF tile once
    (stride-0 partition AP via SWDGE)
  - loop over 8 row-tiles [128, 4096] (2 MiB each): HWDGE DMA in,
    in-place vector-engine multiply against the diag tile, HWDGE DMA out
  - tile pool with multiple bufs so in/compute/out fully overlap
"""

from contextlib import ExitStack

import numpy as np

import bass_rust
import concourse.bass as bass
import concourse.mybir as mybir
from concourse.bass_utils import run_bass_kernel_spmd
from concourse.tile import TileContext
from concourse.vector_clock import ScopedClock


def _split_multiwait_instructions(nc):
    """Walrus codegen in this container accepts at most ONE sync-wait per
    instruction ("Too many sync wait commands", CoreV3GenImpl setupSyncWait).
    Tile's semaphore assignment freely attaches several.  Rewrite every
    basic block: excess waits move onto same-engine NOPs inserted
    immediately before the overloaded instruction (the engine stalls on the
    NOPs first, so the synchronization semantics are identical).
    """
    fn = nc.m.functions[0]
    for bb in fn.blocks:
        insts = list(bb.instructions)
        out = []
        changed = False
        for inst in insts:
            si = inst.sync_info
            waits = list(si.on_wait) if si and si.on_wait else []
            if len(waits) > 1:
                changed = True
                eng = nc.engines[inst.engine]
                for w in waits[:-1]:
                    nop = eng.nop()
                    # eng.nop() appended itself to the current bb; steal it.
                    cur = nc.cur_bb.bb
                    lst = list(cur.instructions)
                    assert lst[-1].name == nop.ins.name
                    cur.instructions = lst[:-1]
                    stolen = nop.ins
                    stolen.sync_info = bass_rust.SyncInfo(on_wait=[w], on_update=[])
                    out.append(stolen)
                si.on_wait = waits[-1:]
            out.append(inst)
        if changed:
            bb.instructions = out


def _patched_drain_and_barrier(self, tick_clock, wait_clock):
    """Replacement for TileContext._drain_and_barrier.

    Identical to stock, plus: runs _split_multiwait_instructions at the end
    (at that point every instruction, including this tail, is committed to
    the basic blocks and no further sync rewriting happens).
    """
    nc = self.nc
    drain_inst = nc.sync.drain()
    wait_clock.add_sem_waits(
        drain_inst.ins, ScopedClock({None: tick_clock.global_clock})
    )
    nc.all_engine_barrier()
    popped = nc._tile_sem_poison_stack.pop()
    assert popped is self._sem_poison
    nc.clear_and_free_semaphores(list(self.sems.allocated().values()))
    nc.all_engine_barrier()
    _split_multiwait_instructions(nc)


TileContext._drain_and_barrier = _patched_drain_and_barrier

BATCH = 8192
SIZE = 4096
N_CORES = 8
ROWS = BATCH // N_CORES  # 1024 rows per core
P = 128
N_TILES = ROWS // P  # 8

_CACHE: dict = {}


def _build() -> bass.Bass:
    nc = bass.Bass("TRN2")
    x = nc.dram_tensor("x", [ROWS, SIZE], mybir.dt.float32, kind="ExternalInput")
    dg = nc.dram_tensor("diagonal", [SIZE], mybir.dt.float32, kind="ExternalInput")
    out = nc.dram_tensor("out", [ROWS, SIZE], mybir.dt.float32, kind="ExternalOutput")

    with TileContext(nc) as tc:
        with (
            ExitStack() as es,
            tc.tile_pool(name="const", bufs=1) as cpool,
            tc.tile_pool(name="psum", bufs=8, space="PSUM") as ppool,
            tc.tile_pool(name="work", bufs=N_TILES) as wpool,
        ):
            # Broadcast the diagonal across all 128 partitions via the (idle)
            # tensor engine: ones[1,128].T @ diag[1,512] -> PSUM[128,512],
            # then DVE copies PSUM -> SBUF.  Only 16 KiB of HBM traffic vs
            # 2 MiB for a stride-0 broadcast DMA riding the load ring.
            ones = cpool.tile([1, P], mybir.dt.float32)
            nc.vector.memset(ones[:], 1.0)
            diag1 = cpool.tile([1, SIZE], mybir.dt.float32)
            nc.sync.dma_start(out=diag1[:], in_=dg[:].partition_broadcast(1))
            dtile = cpool.tile([P, SIZE], mybir.dt.float32)
            MMN = 512  # one PSUM bank of fp32
            for j in range(SIZE // MMN):
                pt = ppool.tile([P, MMN], mybir.dt.float32)
                nc.tensor.matmul(
                    out=pt[:],
                    lhsT=ones[:],
                    rhs=diag1[:, j * MMN : (j + 1) * MMN],
                    start=True,
                    stop=True,
                )
                nc.vector.tensor_copy(dtile[:, j * MMN : (j + 1) * MMN], pt[:])
            # All loads issue first, split across the two HWDGE rings (SP
            # and ACT), so both rings stream input at full rate with no
            # store-waits ahead of them in ring FIFO.  Stores follow behind
            # on the same rings; by the time the rings drain down to a
            # store, its tensor_mul has long completed.
            tiles = []
            for i in range(N_TILES):
                xt = wpool.tile([P, SIZE], mybir.dt.float32)
                eng = nc.sync if i % 2 == 0 else nc.scalar
                eng.dma_start(out=xt[:], in_=x[i * P : (i + 1) * P, :])
                tiles.append(xt)
            for i, xt in enumerate(tiles):
                nc.vector.tensor_mul(xt[:], xt[:], dtile[:])
                eng = nc.scalar if i % 2 == 0 else nc.sync
                eng.dma_start(out=out[i * P : (i + 1) * P, :], in_=xt[:])
    return nc


def kernel(x: np.ndarray, diagonal: np.ndarray) -> np.ndarray:
    if "nc" not in _CACHE:
        _CACHE["nc"] = _build()
    nc = _CACHE["nc"]

    x = np.ascontiguousarray(np.asarray(x, dtype=np.float32))
    diagonal = np.ascontiguousarray(np.asarray(diagonal, dtype=np.float32))

    shards = np.split(x, N_CORES, axis=0)
    in_maps = [{"x": s, "diagonal": diagonal} for s in shards]
    res = run_bass_kernel_spmd(nc, in_maps, list(range(N_CORES))).results
    return np.concatenate([r["out"] for r in res], axis=0)
